# revision 35
# baseline (speedup 1.0000x reference)
"""Sliding-window local attention (KeOps ranges) on 8 Trainium2 cores.

Problem: B=4 H=16 T=4096 D=64, query block w=128 attends keys
[128(i-1), 128(i+1)) clamped to [0, T).  Softmax over the 256-key window,
out = attn @ V.  Only block 0 has out-of-range keys (its lower half), so
masking reduces to skipping that half-block.

Sharding: batch*head (64 pairs) split across 8 cores, 8 heads per core.

Per-core kernel (all matmuls fp16, fp32 PSUM accumulation), organized as
64 "units" (pair, group-of-4-blocks, head) in a 4-deep software pipeline
so the PE never sits idle between the QK and AV phases:

  segment u:  PE: QK matmuls of unit u interleaved with AV matmuls of
              unit u-4 (the AV LDWEIGHTS hide under the long QK matmuls,
              and the tensor engine stays fed so the HAM clock gate keeps
              it at the warm 2.4 GHz state);
              ACT or DVE (alternating by head): exp of unit u-1;
              fp32->fp16 convert of av(u-5) emitted FIRST each segment so
              the av-buffer chain AV(u)->convert(u)->AV(u+2) never waits
              behind an exp on the in-order engine queues.

  - Host pre-arranges SBUF-native layouts (contiguous line-rate DMAs):
      qt/kt: [128, T] per head pair (two heads' [64, T] Q^T/K^T stacked)
      vo:    [128, nblk*(D+1)] per head = exp(m) * [V | 1] blocks
  - S^T[k, q] = K_blk @ Q_blk^T key-major as in the slot scheme below;
    one [128, 8, 128] fp32 PSUM tile (2 banks) per unit, bufs=3.
  - exp: even heads on ACT (exact, scale=1/8, bias=ln(1/16) to keep fp16
    in range), odd heads on DVE via a Schraudolph fast-exp: one
    TensorScalar (s*A + B) -> int16 whose bits ARE the fp16 exp value.
    (rel-err of the fast exp ~2% rms; overall output err ~1.2%.)
  - AV: matmul(lhsT=E^T slot, rhs=vo block) accumulating the two window
    halves into [128, GRP, 65] PSUM (1 bank, bufs=2).
  - Normalization happens on the HOST: the kernel ships fp16
    [numerator | denominator] (65 cols/block) and the host divides.
    On-device work per unit is one fp32->fp16 copy (ACT or DVE).
"""

import numpy as np
from contextlib import ExitStack

import concourse.mybir as mybir
import concourse.tile as tile
from concourse import bacc
from concourse.bass_utils import run_bass_kernel_spmd

B, H, T, D = 4, 16, 4096, 64
W = 128                       # query/key block width
NCORES = 8
HPC = (B * H) // NCORES       # heads per core = 8
NPAIR = HPC // 2              # head pairs per core = 4
GRP = 4                       # query blocks per unit
NBLK = T // W                 # 32
NGRP = NBLK // GRP            # 8
FP16 = mybir.dt.float16
FP32 = mybir.dt.float32
I16 = mybir.dt.int16

# Slot permutation inside one unit's [128, 8, 128] PSUM tile.  Logical
# half-blocks, key-major: key block g0-1+x pairs with query blocks
# (g0-1+x, g0+x).  Slots arranged so every N=256 matmul write stays
# inside a single 2KB PSUM bank (slots 0-3 bank A, 4-7 bank B):
#   bank A slots: (kg0-1,qg0) (kg0,qg0) (kg0,qg0+1) (kg0+3,qg0+3)
#   bank B slots: (kg0+1,qg0+1) (kg0+1,qg0+2) (kg0+2,qg0+2) (kg0+2,qg0+3)
SLOT = [(0, 1), (2, 4), (5, 6), (7, 3)]
# Score matmuls: (key offset dk from g0, first slot, n_query_blocks)
SMM = [(-1, 0, 1), (0, 1, 2), (1, 4, 2), (2, 6, 2), (3, 3, 1)]

# Schraudolph fast-exp constants (DVE): i16 = round(s * A + B) has the
# bit pattern of fp16(exp(s/8)).  A = 0.125*log2(e)*2^10; B tuned to
# minimize rms relative error over s/8 in [-6, 6].  fp16 overflow of the
# AV numerator/denominator is avoided by scaling V (and the ones column)
# by 1/16 on the host instead of scaling the exp.
A_SCH = 0.125 * 1.4426950408889634 * 1024.0
B_SCH = 15300.25


def build_nc(t=T, npair=NPAIR, grp=GRP):
    """Build the single-core Bass program (SPMD across 8 cores)."""
    nblk = t // W
    ngrp = nblk // grp
    hpc = npair * 2
    nu = npair * ngrp * 2      # pipeline units
    nc = bacc.Bacc("TRN2", debug=False, enable_asserts=False)
    qtd = nc.dram_tensor("qt", [npair * W, t], FP16, kind="ExternalInput").ap()
    ktd = nc.dram_tensor("kt", [npair * W, t], FP16, kind="ExternalInput").ap()
    vod = nc.dram_tensor("vo", [hpc * W, nblk * (D + 1)], FP16,
                         kind="ExternalInput").ap()
    ood = nc.dram_tensor("o", [hpc * W, nblk * (D + 1)], FP16,
                         kind="ExternalOutput").ap()

    Exp = mybir.ActivationFunctionType.Exp
    Mult = mybir.AluOpType.mult
    Add = mybir.AluOpType.add

    def unit(u):
        pr, r = divmod(u, ngrp * 2)
        g, e = divmod(r, 2)
        return pr, g, e

    with tile.TileContext(nc) as tc, ExitStack() as ctx:
        qk = ctx.enter_context(tc.tile_pool(name="qk", bufs=2))
        vp = ctx.enter_context(tc.tile_pool(name="vp", bufs=2))
        ep = ctx.enter_context(tc.tile_pool(name="ep", bufs=6))
        osp = ctx.enter_context(tc.tile_pool(name="osp", bufs=2))
        stp = ctx.enter_context(tc.tile_pool(name="stp", bufs=3, space="PSUM"))
        avp = ctx.enter_context(tc.tile_pool(name="avp", bufs=2, space="PSUM"))

        qts, kts, vts, osts = {}, {}, {}, {}
        sts, ets, avs = {}, {}, {}

        def load_pair(pr):
            qt = qk.tile([W, t], FP16, tag="qt", name=f"qt{pr}")
            kt = qk.tile([W, t], FP16, tag="kt", name=f"kt{pr}")
            qts[pr], kts[pr] = qt, kt
            for e in range(2):
                h = 2 * pr + e
                vts[(pr, e)] = vp.tile([W, nblk, D + 1], FP16, tag=f"v{e}",
                                       name=f"vt{pr}_{e}")
                osts[(pr, e)] = osp.tile([W, nblk, D + 1], FP16, tag=f"os{e}",
                                         name=f"ost{pr}_{e}")
            # graduated chunks for pair 0 so the first QK can start ~1.5us
            # in; v-block loads interleaved early (AV needs them soon)
            bounds = ([0, 256, 512, 1024, 2048, 4096] if pr == 0
                      else [0, 2048, 4096])
            vt_after = {2: 0, 3: 1} if pr == 0 else {1: 0, 2: 1}
            for c in range(len(bounds) - 1):
                sl = slice(bounds[c], bounds[c + 1])
                nc.sync.dma_start(out=qt[:, sl], in_=qtd[pr * W:(pr + 1) * W, sl])
                nc.sync.dma_start(out=kt[:, sl], in_=ktd[pr * W:(pr + 1) * W, sl])
                if c + 1 in vt_after:
                    hf = vt_after[c + 1]
                    half = slice(hf * (nblk // 2) * (D + 1),
                                 (hf + 1) * (nblk // 2) * (D + 1))
                    hb = slice(hf * (nblk // 2), (hf + 1) * (nblk // 2))
                    for e in range(2):
                        h = 2 * pr + e
                        nc.sync.dma_start(
                            out=vts[(pr, e)][:, hb, :],
                            in_=vod[h * W:(h + 1) * W, half].rearrange(
                                "p (n d) -> p n d", d=D + 1))

        def emit_exp(u):
            pr, g, e = unit(u)
            et = ep.tile([W, 2 * grp, W], FP16, tag="et", name="et")
            ets[u] = et
            # strict alternation: exactly one exp per segment per engine,
            # so exp latency (1.1-1.2us > segment) pipelines across the
            # two engines instead of serializing on one
            # g==0: slot 0 (key block -1) is PSUM garbage and never read
            # by AV -- skip it (N=896 instead of 1024)
            sl = slice(1, 2 * grp) if g == 0 else slice(0, 2 * grp)
            if e == 0:
                nc.scalar.activation(et[:, sl, :], sts[u][:, sl, :], Exp,
                                     scale=0.125)
            else:
                nc.vector.tensor_scalar(et[:, sl, :].bitcast(I16),
                                        sts[u][:, sl, :],
                                        A_SCH, B_SCH, Mult, Add)

        def qk_mms(u):
            """Allocate st(u) and return the list of QK matmul thunks."""
            pr, g, e = unit(u)
            st = stp.tile([W, 2 * grp, W], FP32, tag="st", name="st")
            sts[u] = st
            # slot (block 0, half j=-1) of g==0 is never written nor read
            # by AV; exp of PSUM garbage is harmless (finite-or-inf fp16)
            dsl = slice(D * e, D * (e + 1))
            qt, kt = qts[pr], kts[pr]
            g0 = g * grp
            ms = []
            for dk, s0, nq in SMM:
                j = g0 + dk
                if j < 0:
                    continue
                qb0 = g0 if dk == -1 else j
                def mm(st=st, s0=s0, nq=nq, j=j, qb0=qb0, dsl=dsl, qt=qt, kt=kt):
                    nc.tensor.matmul(
                        st[:, s0:s0 + nq, :],
                        kt[dsl, W * j:W * (j + 1)],
                        qt[dsl, W * qb0:W * (qb0 + nq)],
                        start=True, stop=True)
                ms.append(mm)
            return ms

        def av_mms(u):
            """Allocate av(u) and return AV accumulation-pair thunks."""
            pr, g, e = unit(u)
            av = avp.tile([W, grp, D + 1], FP32, tag="av", name="av")
            avs[u] = av
            vt, et = vts[(pr, e)], ets[u]
            g0 = g * grp
            pairs = []
            for bi in range(grp):
                i = g0 + bi
                mms = [(SLOT[bi][hi], j)
                       for hi, j in enumerate((i - 1, i)) if j >= 0]
                def pp(av=av, bi=bi, mms=mms, et=et, vt=vt):
                    for x, (s, j) in enumerate(mms):
                        nc.tensor.matmul(
                            av[:, bi, :], et[:, s, :], vt[:, j, :],
                            start=(x == 0), stop=(x == len(mms) - 1))
                pairs.append(pp)
            return pairs

        def emit_convert(u):
            pr, g, e = unit(u)
            g0 = g * grp
            dst = osts[(pr, e)][:, g0:g0 + grp, :]
            # ~40 converts on ACT / 24 on DVE (empirically best balance)
            if u % 8 < 5:
                nc.scalar.copy(dst, avs[u][:])
            else:
                nc.vector.tensor_copy(dst, avs[u][:])

        def emit_out_chunk(pr, e, g_done):
            """Emit the output chunk ending at group g_done (inclusive)."""
            h = 2 * pr + e
            # chunk boundaries: every 4 groups; the very last head drains
            # per-group so the tail is short
            if pr == npair - 1:
                step = 1 if e == 1 else 2
            else:
                step = 4
            if (g_done + 1) % step != 0:
                return
            b0, b1 = (g_done + 1 - step) * grp, (g_done + 1) * grp
            sl = slice(b0 * (D + 1), b1 * (D + 1))
            nc.sync.dma_start(
                out=ood[h * W:(h + 1) * W, sl].rearrange(
                    "p (n d) -> p n d", d=D + 1),
                in_=osts[(pr, e)][:, b0:b1, :])

        # AV lags QK by 4 segments so exp(u) has a ~3-segment window and
        # AV matmuls are always dependency-ready when the scheduler
        # reaches them (it then keeps the qk/av interleave, hiding the
        # AV LDWEIGHTS under the long QK matmuls).  Convert lags 5.
        LAG = 4
        for s in range(nu + LAG + 1):
            cur = s if s < nu else None
            if cur is not None:
                pr, g, e = unit(cur)
                if cur == 0:
                    load_pair(0)
                if g == 4 and e == 0 and pr + 1 < npair:
                    load_pair(pr + 1)
            # convert FIRST: on its engine's in-order queue it must not
            # sit behind this segment's exp (the av-buffer chain
            # AV(u) -> convert(u) -> AV(u+2) would then absorb the exp
            # latency and serialize the whole pipeline)
            if s >= LAG + 1:
                emit_convert(s - LAG - 1)
            if 1 <= s <= nu:
                emit_exp(s - 1)
            qs = qk_mms(cur) if cur is not None else []
            avps = av_mms(s - LAG) if LAG <= s < nu + LAG else []
            # front-load the QK matmuls so exp(cur) can start as early as
            # possible (the exp->AV lockstep latency paces the pipeline);
            # the two AV pairs interleaved at the end still break up the
            # LDWEIGHTS-only run
            for q in qs:
                q()
            for a in avps:
                a()
            if s >= LAG + 1:
                pr3, g3, e3 = unit(s - LAG - 1)
                emit_out_chunk(pr3, e3, g3)
    nc.compile()
    return nc


_NC = None


def _get_nc():
    global _NC
    if _NC is None:
        _NC = build_nc()
    return _NC


def make_in_maps(query_layer, key_layer, value_layer, attention_mask):
    q = np.asarray(query_layer)
    k = np.asarray(key_layer)
    v = np.asarray(value_layer)
    m = np.asarray(attention_mask, dtype=np.float32)
    f16 = np.float16
    qf = q.reshape(B * H, T, D)
    kf = k.reshape(B * H, T, D)
    em = np.exp(m)                                   # [B, T] per-key mask factor
    in_maps = []
    for c in range(NCORES):
        sl = slice(c * HPC, (c + 1) * HPC)
        b = (c * HPC) // H
        qc = (qf[sl].astype(f16).reshape(NPAIR, 2, T, D)
              .transpose(0, 1, 3, 2).reshape(NPAIR * W, T))
        kc = (kf[sl].astype(f16).reshape(NPAIR, 2, T, D)
              .transpose(0, 1, 3, 2).reshape(NPAIR * W, T))
        vc = np.empty((HPC, T, D + 1), np.float32)
        vc[:, :, :D] = v.reshape(B * H, T, D)[sl] * (em[b][None, :, None] / 16)
        vc[:, :, D] = em[b][None, :] / 16
        voc = (vc.astype(f16).reshape(HPC, NBLK, W, D + 1)
               .transpose(0, 2, 1, 3).reshape(HPC * W, NBLK * (D + 1)))
        in_maps.append({
            "qt": np.ascontiguousarray(qc),
            "kt": np.ascontiguousarray(kc),
            "vo": np.ascontiguousarray(voc),
        })
    return in_maps


def run(inputs, trace=False):
    """Run on the 8 cores; returns (full_output, BassKernelResults)."""
    in_maps = make_in_maps(**inputs)
    nc = _get_nc()
    res = run_bass_kernel_spmd(
        nc, in_maps, core_ids=list(range(NCORES)), trace=trace
    )
    out = np.empty((B * H, T, D), np.float32)
    for c in range(NCORES):
        oc = res.results[c]["o"].reshape(HPC, W, NBLK, D + 1)
        oc = oc.transpose(0, 2, 1, 3).astype(np.float32)  # [HPC, NBLK, W, 65]
        out[c * HPC:(c + 1) * HPC] = (
            oc[..., :D] / oc[..., D:]).reshape(HPC, T, D)
    return out.reshape(B, H, T, D), res


def kernel(query_layer, key_layer, value_layer, attention_mask):
    out, _ = run({
        "query_layer": query_layer,
        "key_layer": key_layer,
        "value_layer": value_layer,
        "attention_mask": attention_mask,
    })
    return out
